# revision 20
# baseline (speedup 1.0000x reference)
"""Trainium2 Bass kernel for nn_EnhancedPatchMoE (moe_routing).

Data-parallel: 8 images/core x 8 cores, no collectives. Activations are
feature-major ([D, tokens]) in SBUF. The transformer runs in exact fp32 (so
the MoE top-2 routing decisions match the fp32 reference), the dense MoE and
seg head run in fp32r (full-rate TensorE). Attention computes transposed
scores so the softmax denominator falls out of the AV matmul via an extra
ones-column of V; biases enter through ACT-evacuation or ones-row matmuls.
"""

import numpy as np

import concourse.bass as bass
import concourse.mybir as mybir
import concourse.tile as tile
from concourse import bacc
from concourse.bass_utils import run_bass_kernel_spmd

F32 = mybir.dt.float32
F32R = mybir.dt.float32r
AX = mybir.AxisListType
ALU = mybir.AluOpType
AF = mybir.ActivationFunctionType

NCORE = 8
B, C, H, W = 64, 3, 160, 160
PS, D, NH, NE, NL, NCH, HID = 16, 768, 12, 8, 6, 1, 1536
BC = B // NCORE
L = 100
T = BC * L
KD = D // 128
DH = D // NH
DFF = 4 * D
TEMP, EPS = 0.1, 1e-5
CH = ((0, 400), (400, 400))

_CACHE = {}


def _build():
    nc = bacc.Bacc("TRN2", target_bir_lowering=False, debug=False,
                   num_devices=NCORE)

    def din(name, shape, dt=F32):
        return nc.dram_tensor(name, list(shape), dt, kind="ExternalInput").ap()

    def dout(name, shape, dt=F32):
        return nc.dram_tensor(name, list(shape), dt, kind="ExternalOutput").ap()

    imagesr = din("imagesr", [D, T])   # host pre-patchified [cpq, (b h w)]
    wpT = din("wpT", [D, D])
    bp = din("bp", [D])
    edsT = din("edsT", [D, NE])
    epaT = din("epaT", [D, L])
    eim = din("eim", [1000, D])
    ohds = din("ohds", [NE, T])
    ohpa = din("ohpa", [L, T])
    ohim = din("ohim", [1000, T])
    w1aT = din("w1aT", [D, D])
    w1bT = din("w1bT", [D, D])
    w1cT = din("w1cT", [D, D])
    b1p = din("b1p", [D])
    w2pT = din("w2pT", [D, D])
    b2p = din("b2p", [D])
    lw = []
    for l in range(NL):
        lw.append({k: din(f"L{l}_{k}", s) for k, s in [
            ("qkT", [D, 2 * D]), ("vT", [D, D]), ("bqk", [2 * D]), ("bv", [D]),
            ("woT", [D, D]), ("bo", [D]), ("g1", [D]), ("c1", [D]),
            ("w1T", [D, DFF]), ("b1", [DFF]), ("w2T", [DFF, D]), ("b2", [D]),
            ("g2", [D]), ("c2", [D])]})
    wgT = din("wgT", [D, NE])
    bg = din("bg", [NE])
    ew = []
    for e in range(NE):
        ew.append({
            "w1T": din(f"E{e}_w1T", [D, HID], F32R),
            "b1": din(f"E{e}_b1", [HID]),
            "w2T": din(f"E{e}_w2T", [HID, HID], F32R),
            "b2": din(f"E{e}_b2", [HID]),
            "w3T": din(f"E{e}_w3T", [HID, D], F32R),
            "b3": din(f"E{e}_b3", [D])})
    ws1T = din("ws1T", [D, 384], F32R)
    bs1 = din("bs1", [384])
    ws2c = din("ws2c", [384], F32R)
    bs2 = din("bs2", [1])
    rwT = din("rwT", [10, 160])
    rhT = din("rhT", [10, 160])
    ident = din("ident", [L, L])
    posmask = din("posmask", [BC, L, L])
    validc = din("validc", [L, BC])

    seg_out = dout("seg_out", [BC, NCH, H, W])
    closs_out = dout("closs_out", [1, BC])
    lb_out = dout("lb_out", [BC, L, 2])

    with tile.TileContext(nc) as tc:
        import contextlib
        with contextlib.ExitStack() as ctx:
            pc = ctx.enter_context(tc.tile_pool(name="const", bufs=1))
            pb = ctx.enter_context(tc.tile_pool(name="bias", bufs=1))
            pt = ctx.enter_context(tc.tile_pool(name="tmp", bufs=2))
            pp = ctx.enter_context(tc.tile_pool(name="ps", bufs=2, space="PSUM"))
            pdram = ctx.enter_context(tc.tile_pool(name="dram", bufs=1, space="DRAM"))
            import contextlib as _cl
            tctx = _cl.ExitStack()
            pw = tctx.enter_context(tc.tile_pool(name="wts", bufs=2))
            pwv = tctx.enter_context(tc.tile_pool(name="wtsv", bufs=1))
            pw2 = tctx.enter_context(tc.tile_pool(name="wts2", bufs=1))
            pact = tctx.enter_context(tc.tile_pool(name="pact", bufs=1))

            def psum(tag):
                return pp.tile([128, 512], F32, tag=tag, name=tag)

            def col_load(vec, n, tag):
                t = pb.tile([128, n // 128], F32, tag=tag, name=tag)
                nc.sync.dma_start(t[:], vec.rearrange("(t p) -> p t", p=128))
                return t

            def wload(wT, m, kt=KD, tag="wk", dt=F32):
                t = pw.tile([128, kt * 128], dt, tag=tag, name=tag)
                nc.sync.dma_start(
                    t[:].rearrange("p (t f) -> p t f", t=kt),
                    wT[:, m * 128:(m + 1) * 128].rearrange(
                        "(t p) f -> p t f", p=128))
                return t

            # persistent state ------------------------------------------------
            ones_col = pc.tile([128, 1], F32, tag="ones_col", name="ones_col")
            nc.gpsimd.memset(ones_col[:], 1.0)
            ones_row = pc.tile([1, L], F32, tag="ones_row", name="ones_row")
            nc.gpsimd.memset(ones_row[:], 1.0)
            # all big activation tiles are f32r-typed; fp32-exact stages use
            # .bitcast(F32) views of them
            mk = lambda pool, pre, cnt: [
                pool.tile([128, T], F32, tag=f"{pre}{m}", name=f"{pre}{m}")[:]
                for m in range(cnt)]
            xa, xb = mk(pact, "xa", KD), mk(pact, "xb", KD)
            big = mk(pact, "bg", 12)
            osq, yt = mk(pact, "oq", KD), mk(pact, "yt", KD)
            vst = [pc.tile([L, NH * (DH + 1)], F32, tag=f"v{i}", name=f"v{i}")
                   for i in range(BC)]
            for i in range(BC):
                nc.gpsimd.memset(
                    vst[i][:].rearrange("p (h d) -> p h d", h=NH)
                    [:, :, DH:DH + 1], 1.0)
            row_a = pc.tile([1, T], F32, tag="row_a", name="row_a")
            row_b = pc.tile([1, T], F32, tag="row_b", name="row_b")
            row_c = pc.tile([1, T], F32, tag="row_c", name="row_c")
            eps_t = pc.tile([1, 1], F32, tag="eps", name="eps")
            nc.gpsimd.memset(eps_t[:], EPS)
            mbc = pc.tile([128, T], F32, tag="mbc", name="mbc")
            rbc = pc.tile([128, T], F32, tag="rbc", name="rbc")

            def stat_rows(src, dst_row, scale, tag="st"):
                for o, n in CH:
                    p = pp.tile([1, 512], F32, tag=tag, name=tag)
                    for k in range(KD):
                        nc.tensor.matmul(p[:, :n], ones_col[:],
                                         src[k][:, o:o + n],
                                         start=(k == 0), stop=(k == KD - 1))
                    nc.scalar.mul(dst_row[:, o:o + n], p[:, :n], scale)

            def layernorm(y, sq, dst, gcol, ccol):
                # y: f32 views [128,T] x6 ; dst may alias y (in-place)
                mrow, srow, tmpr = row_a[:], row_b[:], row_c[:]
                for m in range(KD):
                    nc.scalar.activation(sq[m], y[m], AF.Square)
                stat_rows(y, mrow, 1.0 / D)
                stat_rows(sq, srow, 1.0 / D)
                nc.vector.tensor_mul(tmpr, mrow, mrow)
                nc.vector.tensor_sub(srow, srow, tmpr)
                nc.scalar.activation(tmpr, srow, AF.Sqrt, bias=eps_t[:])
                nc.vector.reciprocal(srow, tmpr)
                nc.gpsimd.partition_broadcast(mbc[:], mrow)
                nc.gpsimd.partition_broadcast(rbc[:], srow)
                for m in range(KD):
                    lnt = pt.tile([128, T], F32, tag="cmb", name="cmb")
                    nc.vector.tensor_sub(lnt[:], y[m], mbc[:])
                    nc.vector.tensor_mul(lnt[:], lnt[:], rbc[:])
                    nc.vector.scalar_tensor_tensor(
                        dst[m], lnt[:], gcol[:, m:m + 1],
                        ccol[:, m:m + 1].to_broadcast((128, T)),
                        op0=ALU.mult, op1=ALU.add)

            # ================= patch embed + position MLP ====================
            xp = big[6:12]
            for kt in range(KD):
                nc.sync.dma_start(xp[kt][:],
                                  imagesr[kt * 128:(kt + 1) * 128, :])
            gim = osq
            for m in range(KD):
                for o, n in CH:
                    p = psum("g")
                    for kt in range(8):
                        et = pw.tile([125, 128], F32, tag="eim", name="eim")
                        nc.sync.dma_start(
                            et[:], eim[kt * 125:(kt + 1) * 125,
                                       m * 128:(m + 1) * 128])
                        ot = pwv.tile([125, 400], F32, tag="wv", name="wv")
                        nc.sync.dma_start(ot[:],
                                          ohim[kt * 125:(kt + 1) * 125,
                                               o:o + n])
                        nc.tensor.matmul(p[:, :n], et[:], ot[:, :n],
                                         start=(kt == 0), stop=(kt == 7))
                    nc.scalar.copy(gim[m][:, o:o + n], p[:, :n])
            eds_sb = pc.tile([128, KD * NE], F32, tag="mixa", name="mixa")
            nc.sync.dma_start(eds_sb[:].rearrange("p (t f) -> p t f", t=KD),
                              edsT.rearrange("(t p) f -> p t f", p=128))
            epa_sb = pc.tile([128, KD * L], F32, tag="mixb", name="mixb")
            nc.sync.dma_start(epa_sb[:].rearrange("p (t f) -> p t f", t=KD),
                              epaT.rearrange("(t p) f -> p t f", p=128))
            ads = pc.tile([NE, D], F32, tag="ads", name="ads")
            apa = pc.tile([L, D], F32, tag="apa", name="apa")
            for n2 in range(2):
                wa = pwv.tile([128, KD * 384], F32, tag="wv", name="wv")
                nc.sync.dma_start(
                    wa[:].rearrange("p (t f) -> p t f", t=KD),
                    w1aT[:, n2 * 384:(n2 + 1) * 384].rearrange(
                        "(t p) f -> p t f", p=128))
                p = psum("g")
                for k in range(KD):
                    nc.tensor.matmul(p[:NE, :384],
                                     eds_sb[:, k * NE:(k + 1) * NE],
                                     wa[:, k * 384:(k + 1) * 384],
                                     start=(k == 0), stop=(k == KD - 1))
                nc.scalar.copy(ads[:, n2 * 384:(n2 + 1) * 384], p[:NE, :384])
                wc = pwv.tile([128, KD * 384], F32, tag="wv", name="wv")
                nc.sync.dma_start(
                    wc[:].rearrange("p (t f) -> p t f", t=KD),
                    w1cT[:, n2 * 384:(n2 + 1) * 384].rearrange(
                        "(t p) f -> p t f", p=128))
                p = psum("g")
                for k in range(KD):
                    nc.tensor.matmul(p[:L, :384],
                                     epa_sb[:, k * L:(k + 1) * L],
                                     wc[:, k * 384:(k + 1) * 384],
                                     start=(k == 0), stop=(k == KD - 1))
                nc.scalar.copy(apa[:, n2 * 384:(n2 + 1) * 384], p[:L, :384])
            ohds_sb = pc.tile([NE, T], F32, tag="ohds", name="ohds")
            nc.sync.dma_start(ohds_sb[:], ohds)
            ohpa_sb = pc.tile([L, T], F32, tag="ohpa", name="ohpa")
            nc.sync.dma_start(ohpa_sb[:], ohpa)
            b1p_c = col_load(b1p, D, "cl_misc")
            hT = big[0:6]
            for m in range(KD):
                wb = wload(w1bT, m)
                for o, n in CH:
                    p = psum("g")
                    for k in range(KD):
                        nc.tensor.matmul(p[:, :n], wb[:, k * 128:(k + 1) * 128],
                                         gim[k][:, o:o + n],
                                         start=(k == 0), stop=False)
                    nc.tensor.matmul(p[:, :n], ads[:, m * 128:(m + 1) * 128],
                                     ohds_sb[:, o:o + n],
                                     start=False, stop=False)
                    nc.tensor.matmul(p[:, :n], apa[:, m * 128:(m + 1) * 128],
                                     ohpa_sb[:, o:o + n],
                                     start=False, stop=True)
                    nc.scalar.activation(hT[m][:, o:o + n], p[:, :n], AF.Relu,
                                         bias=b1p_c[:, m:m + 1])
            bp_c = col_load(bp, D, "cl_misc")
            patchesT = yt
            for m in range(KD):
                wp_sb = wload(wpT, m)
                for o, n in CH:
                    p = psum("g")
                    for k in range(KD):
                        nc.tensor.matmul(p[:, :n],
                                         wp_sb[:, k * 128:(k + 1) * 128],
                                         xp[k][:, o:o + n],
                                         start=(k == 0), stop=(k == KD - 1))
                    nc.scalar.activation(patchesT[m][:, o:o + n], p[:, :n],
                                         AF.Identity, bias=bp_c[:, m:m + 1])
            b2p_c = col_load(b2p, D, "cl_misc")
            for m in range(KD):
                w2_sb = wload(w2pT, m)
                for o, n in CH:
                    p = psum("g")
                    for k in range(KD):
                        nc.tensor.matmul(p[:, :n],
                                         w2_sb[:, k * 128:(k + 1) * 128],
                                         hT[k][:, o:o + n],
                                         start=(k == 0), stop=(k == KD - 1))
                    nc.vector.scalar_tensor_tensor(
                        xa[m][:, o:o + n], p[:, :n], b2p_c[:, m:m + 1],
                        patchesT[m][:, o:o + n], op0=ALU.add, op1=ALU.add)

            # ========================= transformer ===========================
            for l in range(NL):
                w = lw[l]
                xin = xa if l % 2 == 0 else xb
                xout = xb if l % 2 == 0 else xa
                bqk_c = col_load(w["bqk"], 2 * D, "cl_bqk")
                qk = big
                for m in range(12):
                    wq = wload(w["qkT"], m)
                    for o, n in CH:
                        p = psum("g")
                        for k in range(KD):
                            nc.tensor.matmul(p[:, :n],
                                             wq[:, k * 128:(k + 1) * 128],
                                             xin[k][:, o:o + n],
                                             start=(k == 0), stop=(k == KD - 1))
                        nc.scalar.activation(qk[m][:, o:o + n], p[:, :n],
                                             AF.Identity,
                                             bias=bqk_c[:, m:m + 1])
                bv_r = pb.tile([1, D], F32, tag="bv", name="bv")
                nc.sync.dma_start(bv_r[:], w["bv"][None, :])
                for n2 in range(2):
                    wv = pwv.tile([128, KD * 384], F32, tag="wv", name="wv")
                    nc.sync.dma_start(
                        wv[:].rearrange("p (t f) -> p t f", t=KD),
                        w["vT"][:, n2 * 384:(n2 + 1) * 384].rearrange(
                            "(t p) f -> p t f", p=128))
                    for i in range(BC):
                        p = psum("g")
                        for k in range(KD):
                            nc.tensor.matmul(p[:L, :384],
                                             xin[k][:, i * L:(i + 1) * L],
                                             wv[:, k * 384:(k + 1) * 384],
                                             start=(k == 0), stop=False)
                        nc.tensor.matmul(p[:L, :384], ones_row[:],
                                         bv_r[:, n2 * 384:(n2 + 1) * 384],
                                         start=False, stop=True)
                        nc.scalar.copy(
                            vst[i][:].rearrange("p (h d) -> p h d", h=NH)
                            [:, 6 * n2:6 * n2 + 6, 0:DH],
                            p[:L, :384].rearrange("p (h d) -> p h d", h=6))
                oT = osq
                for i in range(BC):
                    for h in range(NH):
                        mt, ro = 6 + h // 2, (h % 2) * 64
                        qs = qk[h // 2][ro:ro + 64, i * L:(i + 1) * L]
                        ks = qk[mt][ro:ro + 64, i * L:(i + 1) * L]
                        sp = pp.tile([L, L], F32, tag="attn", name="attn")
                        nc.tensor.matmul(sp[:], ks, qs, start=True, stop=True)
                        ex = pt.tile([L, L], F32, tag="scr100", name="scr100")
                        nc.scalar.activation(ex[:], sp[:], AF.Exp, scale=0.125)
                        ap = pp.tile([DH + 1, L], F32, tag="av", name="av")
                        nc.tensor.matmul(
                            ap[:], vst[i][:, h * (DH + 1):(h + 1) * (DH + 1)],
                            ex[:], start=True, stop=True)
                        rc = pt.tile([1, L], F32, tag="rcp", name="rcp")
                        nc.vector.reciprocal(rc[:], ap[DH:DH + 1, :])
                        rb = pt.tile([64, L], F32, tag="rb", name="rb")
                        nc.gpsimd.partition_broadcast(rb[:], rc[:])
                        nc.vector.tensor_mul(
                            oT[h // 2][ro:ro + 64, i * L:(i + 1) * L],
                            ap[0:DH, :], rb[:])
                bo_c = col_load(w["bo"], D, "cl_bo")
                for m in range(KD):
                    wo = wload(w["woT"], m)
                    for o, n in CH:
                        p = psum("g")
                        for k in range(KD):
                            nc.tensor.matmul(p[:, :n],
                                             wo[:, k * 128:(k + 1) * 128],
                                             oT[k][:, o:o + n],
                                             start=(k == 0), stop=(k == KD - 1))
                        nc.vector.scalar_tensor_tensor(
                            yt[m][:, o:o + n], p[:, :n], bo_c[:, m:m + 1],
                            xin[m][:, o:o + n], op0=ALU.add, op1=ALU.add)
                g1_c = col_load(w["g1"], D, "cl_g1")
                c1_c = col_load(w["c1"], D, "cl_c1")
                layernorm(yt, osq, yt, g1_c, c1_c)   # x1 lands in yt
                b1_c = col_load(w["b1"], DFF, "cl_b1")
                b2_c = col_load(w["b2"], D, "cl_b2")
                for o, n in CH:
                    for m in range(24):
                        wf = wload(w["w1T"], m)
                        p = psum("g")
                        for k in range(KD):
                            nc.tensor.matmul(p[:, :n],
                                             wf[:, k * 128:(k + 1) * 128],
                                             yt[k][:, o:o + n],
                                             start=(k == 0), stop=(k == KD - 1))
                        nc.scalar.activation(
                            big[m // 2][:, (m % 2) * 400:(m % 2) * 400 + n],
                            p[:, :n], AF.Relu, bias=b1_c[:, m:m + 1])
                    for m in range(KD):
                        p = psum("g")
                        for hv in range(2):
                            wf2 = pw2.tile([128, 12 * 128], F32, tag="wk2",
                                           name="wk2")
                            nc.sync.dma_start(
                                wf2[:].rearrange("p (t f) -> p t f", t=12),
                                w["w2T"][hv * 1536:(hv + 1) * 1536,
                                         m * 128:(m + 1) * 128].rearrange(
                                    "(t p) f -> p t f", p=128))
                            for kk in range(12):
                                k = hv * 12 + kk
                                nc.tensor.matmul(
                                    p[:, :n], wf2[:, kk * 128:(kk + 1) * 128],
                                    big[k // 2][:, (k % 2) * 400:(k % 2) * 400 + n],
                                    start=(k == 0), stop=(k == 23))
                        nc.vector.scalar_tensor_tensor(
                            osq[m][:, o:o + n], p[:, :n], b2_c[:, m:m + 1],
                            yt[m][:, o:o + n], op0=ALU.add, op1=ALU.add)
                g2_c = col_load(w["g2"], D, "cl_g2")
                c2_c = col_load(w["c2"], D, "cl_c2")
                layernorm(osq, big[0:6], xout, g2_c, c2_c)

            xf = xa if NL % 2 == 0 else xb

            # ===================== gate + top-2 weights ======================
            wg_sb = pc.tile([128, KD * NE], F32, tag="mixa", name="mixa")
            nc.sync.dma_start(wg_sb[:].rearrange("p (t f) -> p t f", t=KD),
                              wgT.rearrange("(t p) f -> p t f", p=128))
            bg_r = pb.tile([1, NE], F32, tag="bg", name="bg")
            nc.sync.dma_start(bg_r[:], bg[None, :])
            id_sb = pc.tile([L, L], F32, tag="mixb", name="mixb")
            nc.sync.dma_start(id_sb[:], ident)
            wrow = pc.tile([NE, T], F32, tag="ohds", name="ohds")
            lbst = pc.tile([L, 2 * BC], F32, tag="lbstage", name="lbstage")
            for i in range(BC):
                p = psum("g")
                for k in range(KD):
                    nc.tensor.matmul(p[:L, :NE], xf[k][:, i * L:(i + 1) * L],
                                     wg_sb[:, k * NE:(k + 1) * NE],
                                     start=(k == 0), stop=False)
                nc.tensor.matmul(p[:L, :NE], ones_row[:], bg_r[:],
                                 start=False, stop=True)
                g = pt.tile([L, NE], F32, tag="gate", name="gate")
                nc.scalar.copy(g[:], p[:L, :NE])
                m1 = pt.tile([L, 1], F32, tag="m1", name="m1")
                nc.vector.tensor_reduce(m1[:], g[:], AX.X, ALU.max)
                oh1 = pt.tile([L, NE], F32, tag="oh1", name="oh1")
                nc.vector.tensor_scalar(out=oh1[:], in0=g[:], scalar1=m1[:],
                                        scalar2=None, op0=ALU.is_equal)
                g2 = pt.tile([L, NE], F32, tag="g2", name="g2")
                nc.vector.scalar_tensor_tensor(g2[:], oh1[:], -1e30, g[:],
                                               op0=ALU.mult, op1=ALU.add)
                m2 = pt.tile([L, 1], F32, tag="m2", name="m2")
                nc.vector.tensor_reduce(m2[:], g2[:], AX.X, ALU.max)
                oh2 = pt.tile([L, NE], F32, tag="oh2", name="oh2")
                nc.vector.tensor_scalar(out=oh2[:], in0=g2[:], scalar1=m2[:],
                                        scalar2=None, op0=ALU.is_equal)
                dm = pt.tile([L, 1], F32, tag="dm", name="dm")
                nc.vector.tensor_sub(dm[:], m2[:], m1[:])
                edm = pt.tile([L, 1], F32, tag="edm", name="edm")
                nc.scalar.activation(edm[:], dm[:], AF.Exp)
                sm = pt.tile([L, 1], F32, tag="sm", name="sm")
                nc.vector.tensor_scalar_add(sm[:], edm[:], 1.0)
                w1v = pt.tile([L, 1], F32, tag="w1v", name="w1v")
                nc.vector.reciprocal(w1v[:], sm[:])
                w2v = pt.tile([L, 1], F32, tag="w2v", name="w2v")
                nc.vector.tensor_mul(w2v[:], edm[:], w1v[:])
                nc.vector.tensor_copy(lbst[:, 2 * i:2 * i + 1], w1v[:])
                nc.vector.tensor_copy(lbst[:, 2 * i + 1:2 * i + 2], w2v[:])
                wg8 = pt.tile([L, NE], F32, tag="wg8", name="wg8")
                nc.vector.tensor_scalar(out=wg8[:], in0=oh2[:], scalar1=w2v[:],
                                        scalar2=None, op0=ALU.mult)
                nc.vector.scalar_tensor_tensor(wg8[:], oh1[:], w1v[:], wg8[:],
                                               op0=ALU.mult, op1=ALU.add)
                tp = pp.tile([NE, L], F32, tag="st", name="st")
                nc.tensor.transpose(tp[:], wg8[:], id_sb[:])
                nc.scalar.copy(wrow[:, i * L:(i + 1) * L], tp[:])
            nc.sync.dma_start(
                lb_out.rearrange("b l w -> l b w"),
                lbst[:].rearrange("l (b w) -> l b w", w=2))

            # ==================== contrastive loss (uses xf) =================
            sdr, rr = row_a[:], row_b[:]
            for m in range(KD):
                nc.scalar.activation(osq[m], xf[m], AF.Square)
            stat_rows(osq, sdr, 1.0)
            nc.scalar.activation(rr, sdr, AF.Sqrt)
            nc.vector.reciprocal(sdr, rr)
            nc.gpsimd.partition_broadcast(rbc[:], sdr)
            fn = osq
            for m in range(KD):
                nc.vector.tensor_mul(fn[m], xf[m], rbc[:])
            vld = pc.tile([L, BC], F32, tag="vld", name="vld")
            nc.sync.dma_start(vld[:], validc)
            ccols = pc.tile([L, BC], F32, tag="ccols", name="ccols")
            for i in range(BC):
                gp = pp.tile([L, L], F32, tag="attn", name="attn")
                for k in range(KD):
                    nc.tensor.matmul(gp[:], fn[k][:, i * L:(i + 1) * L],
                                     fn[k][:, i * L:(i + 1) * L],
                                     start=(k == 0), stop=(k == KD - 1))
                rmx = pt.tile([L, 1], F32, tag="rmx", name="rmx")
                nc.vector.tensor_reduce(rmx[:], gp[:], AX.X, ALU.max)
                nb = pt.tile([L, 1], F32, tag="nb", name="nb")
                nc.scalar.mul(nb[:], rmx[:], -1.0 / TEMP)
                pex = pt.tile([L, L], F32, tag="scr100", name="scr100")
                rsum = pt.tile([L, 1], F32, tag="rsum", name="rsum")
                nc.scalar.activation(pex[:], gp[:], AF.Exp, bias=nb[:],
                                     scale=1.0 / TEMP, accum_out=rsum[:])
                lnr = pt.tile([L, 1], F32, tag="lnr", name="lnr")
                nc.scalar.activation(lnr[:], rsum[:], AF.Ln)
                pmk = pt.tile([L, L], F32, tag="scr100", name="scr100")
                nc.sync.dma_start(pmk[:], posmask[i])
                tgm = pt.tile([L, L], F32, tag="scr100", name="scr100")
                nc.vector.tensor_mul(tgm[:], gp[:], pmk[:])
                tgs = pt.tile([L, 1], F32, tag="tgs", name="tgs")
                nc.vector.tensor_reduce(tgs[:], tgm[:], AX.X, ALU.add)
                ar = pt.tile([L, 1], F32, tag="ar", name="ar")
                nc.vector.scalar_tensor_tensor(
                    ar[:], rmx[:], 1.0 / TEMP, lnr[:], op0=ALU.mult, op1=ALU.add)
                br = pt.tile([L, 1], F32, tag="br", name="br")
                nc.vector.scalar_tensor_tensor(
                    br[:], tgs[:], -1.0 / TEMP, ar[:], op0=ALU.mult, op1=ALU.add)
                nc.vector.tensor_mul(ccols[:, i:i + 1], br[:], vld[:, i:i + 1])
            p = pp.tile([1, 512], F32, tag="st", name="st")
            nc.tensor.matmul(p[:, :BC], ones_col[0:L, :], ccols[:],
                             start=True, stop=True)
            nc.scalar.copy(row_c[0:1, 0:BC], p[:, :BC])
            nc.sync.dma_start(closs_out, row_c[0:1, 0:BC])

            # ============================ MoE ================================
            xdram = pdram.tile([D, T], F32R, tag="xdram", name="xdram")
            for m in range(KD):
                nc.sync.dma_start(xdram[m * 128:(m + 1) * 128, :],
                                  xf[m].bitcast(F32R))
            tctx.close()   # free transformer-era SBUF (fresh f32r locations)
            pwm = ctx.enter_context(tc.tile_pool(name="wtsm", bufs=2))
            pw2m = ctx.enter_context(tc.tile_pool(name="wts2m", bufs=1))
            pm = ctx.enter_context(tc.tile_pool(name="moe", bufs=1))
            mkr = lambda pre, cnt: [
                pm.tile([128, T], F32R, tag=f"{pre}{m}", name=f"{pre}{m}")[:]
                for m in range(cnt)]
            xfr = mkr("xfr", KD)
            for m in range(KD):
                nc.sync.dma_start(xfr[m], xdram[m * 128:(m + 1) * 128, :])
            h1T = mkr("h1t", 12)
            h2T = mkr("h2t", 12)
            acc_r = mkr("accm", KD)
            wbc = mbc
            for e in range(NE):
                we = ew[e]
                nc.sync.dma_start(row_c[:], wrow[e:e + 1, :])
                nc.gpsimd.partition_broadcast(wbc[:], row_c[:])
                be1_c = col_load(we["b1"], HID, "cl_b1")
                for m in range(12):
                    wt = pwm.tile([128, KD * 128], F32R, tag="wkm", name="wkm")
                    nc.sync.dma_start(
                        wt[:].rearrange("p (t f) -> p t f", t=KD),
                        we["w1T"][:, m * 128:(m + 1) * 128].rearrange(
                            "(t p) f -> p t f", p=128))
                    for o, n in CH:
                        p = psum("g")
                        for k in range(KD):
                            nc.tensor.matmul(p[:, :n],
                                             wt[:, k * 128:(k + 1) * 128],
                                             xfr[k][:, o:o + n],
                                             start=(k == 0), stop=(k == KD - 1))
                        nc.scalar.activation(h1T[m][:, o:o + n], p[:, :n],
                                             AF.Relu, bias=be1_c[:, m:m + 1])
                be2_c = col_load(we["b2"], HID, "cl_b2")
                for m in range(12):
                    wt = pw2m.tile([128, 12 * 128], F32R, tag="wk2m",
                                   name="wk2m")
                    nc.sync.dma_start(
                        wt[:].rearrange("p (t f) -> p t f", t=12),
                        we["w2T"][:, m * 128:(m + 1) * 128].rearrange(
                            "(t p) f -> p t f", p=128))
                    for o, n in CH:
                        p = psum("g")
                        for k in range(12):
                            nc.tensor.matmul(p[:, :n],
                                             wt[:, k * 128:(k + 1) * 128],
                                             h1T[k][:, o:o + n],
                                             start=(k == 0), stop=(k == 11))
                        nc.scalar.activation(h2T[m][:, o:o + n], p[:, :n],
                                             AF.Relu, bias=be2_c[:, m:m + 1])
                be3_c = col_load(we["b3"], D, "cl_b3")
                for m in range(KD):
                    wt = pw2m.tile([128, 12 * 128], F32R, tag="wk2m",
                                   name="wk2m")
                    nc.sync.dma_start(
                        wt[:].rearrange("p (t f) -> p t f", t=12),
                        we["w3T"][:, m * 128:(m + 1) * 128].rearrange(
                            "(t p) f -> p t f", p=128))
                    for o, n in CH:
                        p = psum("g")
                        for k in range(12):
                            nc.tensor.matmul(p[:, :n],
                                             wt[:, k * 128:(k + 1) * 128],
                                             h2T[k][:, o:o + n],
                                             start=(k == 0), stop=(k == 11))
                        if e == 0:
                            nc.vector.scalar_tensor_tensor(
                                acc_r[m][:, o:o + n], p[:, :n],
                                be3_c[:, m:m + 1], wbc[:, o:o + n],
                                op0=ALU.add, op1=ALU.mult)
                        else:
                            t = pt.tile([128, T], F32, tag="cmb", name="cmb")
                            nc.vector.scalar_tensor_tensor(
                                t[:, :n], p[:, :n], be3_c[:, m:m + 1],
                                wbc[:, o:o + n], op0=ALU.add, op1=ALU.mult)
                            nc.vector.tensor_add(acc_r[m][:, o:o + n],
                                                 acc_r[m][:, o:o + n],
                                                 t[:, :n])

            # ========================= seg head ==============================
            bs1_c = col_load(bs1, 384, "cl_misc")
            s1 = [pm.tile([128, T], F32R, tag=f"s1_{m}", name=f"s1_{m}")[:]
                  for m in range(3)]
            for m in range(3):
                wt = pwm.tile([128, KD * 128], F32R, tag="wkm", name="wkm")
                nc.sync.dma_start(
                    wt[:].rearrange("p (t f) -> p t f", t=KD),
                    ws1T[:, m * 128:(m + 1) * 128].rearrange(
                        "(t p) f -> p t f", p=128))
                for o, n in CH:
                    p = psum("g")
                    for k in range(KD):
                        nc.tensor.matmul(p[:, :n], wt[:, k * 128:(k + 1) * 128],
                                         acc_r[k][:, o:o + n],
                                         start=(k == 0), stop=(k == KD - 1))
                    nc.scalar.activation(s1[m][:, o:o + n], p[:, :n], AF.Relu,
                                         bias=bs1_c[:, m:m + 1])
            ws2_sb = pb.tile([128, 3], F32R, tag="ws2", name="ws2")
            nc.sync.dma_start(ws2_sb[:], ws2c.rearrange("(t p) -> p t", p=128))
            bs2_r = pb.tile([1, 1], F32, tag="bs2", name="bs2")
            nc.sync.dma_start(bs2_r[:], bs2[None, :])
            segr = row_a[:]
            for o, n in CH:
                p = pp.tile([1, 512], F32, tag="st", name="st")
                for k in range(3):
                    nc.tensor.matmul(p[:, :n], ws2_sb[:, k:k + 1],
                                     s1[k][:, o:o + n],
                                     start=(k == 0), stop=(k == 2))
                nc.scalar.activation(segr[:, o:o + n], p[:, :n], AF.Identity,
                                     bias=bs2_r[:])
            rw_sb = pc.tile([10, 160], F32, tag="rw", name="rw")
            nc.sync.dma_start(rw_sb[:], rwT)
            rh_sb = pc.tile([10, 160], F32, tag="rh", name="rh")
            nc.sync.dma_start(rh_sb[:], rhT)
            for i in range(BC):
                shw = pt.tile([10, 10], F32, tag="bils", name="bils")
                nc.sync.dma_start(
                    shw[:], segr[0:1, i * L:(i + 1) * L].rearrange(
                        "p (h w) -> p h w", w=10))
                tps = pp.tile([10, 10], F32, tag="st", name="st")
                nc.tensor.transpose(tps[:], shw[:], id_sb[0:10, 0:10])
                sbt = pt.tile([10, 10], F32, tag="bils", name="bils")
                nc.scalar.copy(sbt[:], tps[:])
                t1p = pp.tile([80, 512], F32, tag="st", name="st")
                nc.tensor.matmul(t1p[:10, :160], sbt[:], rw_sb[:],
                                 start=True, stop=True)
                t1 = pt.tile([10, 160], F32, tag="bils", name="bils")
                nc.scalar.copy(t1[:], t1p[:10, :160])
                for hf in range(2):
                    op = pp.tile([80, 512], F32, tag="st", name="st")
                    nc.tensor.matmul(op[:, :160],
                                     rh_sb[:, hf * 80:(hf + 1) * 80], t1[:],
                                     start=True, stop=True)
                    ot = pt.tile([80, 160], F32, tag="bils", name="bils")
                    nc.scalar.copy(ot[:], op[:, :160])
                    nc.sync.dma_start(seg_out[i, 0, hf * 80:(hf + 1) * 80, :],
                                      ot[:])

    nc.compile()
    return nc


def _resize_mat(n_in, n_out):
    R = np.zeros((n_out, n_in), np.float64)
    s = n_in / n_out
    for i in range(n_out):
        c = min(max((i + 0.5) * s - 0.5, 0.0), n_in - 1)
        lo = int(np.floor(c))
        hi = min(lo + 1, n_in - 1)
        w = c - lo
        R[i, lo] += 1.0 - w
        R[i, hi] += w
    return R.astype(np.float32)


def kernel(images, dataset_ids, image_ids, params):
    images = np.asarray(images, np.float32)
    dataset_ids = np.asarray(dataset_ids)
    image_ids = np.asarray(image_ids)
    p = {k: (np.asarray(v, np.float32) if not isinstance(v, list) else
             [{k2: np.asarray(v2, np.float32) for k2, v2 in d.items()}
              for d in v]) for k, v in params.items()}

    if "nc" not in _CACHE:
        _CACHE["nc"] = _build()
    nc = _CACHE["nc"]

    tr = lambda a: np.ascontiguousarray(a.T)
    shared = {
        "wpT": tr(p["Wp"].reshape(D, D)), "bp": p["bp"],
        "edsT": tr(p["E_ds"]), "epaT": tr(p["E_pa"]),
        "eim": np.ascontiguousarray(p["E_im"]),
        "w1aT": tr(p["pos_W1"][:, 0:D]), "w1bT": tr(p["pos_W1"][:, D:2 * D]),
        "w1cT": tr(p["pos_W1"][:, 2 * D:3 * D]), "b1p": p["pos_b1"],
        "w2pT": tr(p["pos_W2"]), "b2p": p["pos_b2"],
        "wgT": tr(p["Wg"]), "bg": p["bg"],
        "ws1T": tr(p["Ws1"]), "bs1": p["bs1"],
        "ws2c": np.ascontiguousarray(p["Ws2"][0]), "bs2": p["bs2"],
        "rwT": tr(_resize_mat(10, 160)), "rhT": tr(_resize_mat(10, 160)),
        "ident": np.eye(L, dtype=np.float32),
    }
    for l in range(NL):
        d = p["layers"][l]
        shared.update({
            f"L{l}_qkT": tr(d["Wqkv"][0:2 * D]),
            f"L{l}_vT": tr(d["Wqkv"][2 * D:]),
            f"L{l}_bqk": d["bqkv"][0:2 * D], f"L{l}_bv": d["bqkv"][2 * D:],
            f"L{l}_woT": tr(d["Wo"]), f"L{l}_bo": d["bo"],
            f"L{l}_g1": d["ln1_g"], f"L{l}_c1": d["ln1_b"],
            f"L{l}_w1T": tr(d["W1"]), f"L{l}_b1": d["b1"],
            f"L{l}_w2T": tr(d["W2"]), f"L{l}_b2": d["b2"],
            f"L{l}_g2": d["ln2_g"], f"L{l}_c2": d["ln2_b"]})
    for e in range(NE):
        shared.update({
            f"E{e}_w1T": tr(p["We1"][e]), f"E{e}_b1": p["be1"][e],
            f"E{e}_w2T": tr(p["We2"][e]), f"E{e}_b2": p["be2"][e],
            f"E{e}_w3T": tr(p["We3"][e]), f"E{e}_b3": p["be3"][e]})

    in_maps = []
    for c in range(NCORE):
        ds = dataset_ids[c * BC:(c + 1) * BC]
        im = image_ids[c * BC:(c + 1) * BC]
        ohds_a = np.zeros((NE, T), np.float32)
        ohim_a = np.zeros((1000, T), np.float32)
        ohpa_a = np.zeros((L, T), np.float32)
        t = np.arange(T)
        ohds_a[np.asarray(ds).reshape(-1), t] = 1.0
        ohim_a[np.asarray(im).reshape(-1), t] = 1.0
        ohpa_a[t % L, t] = 1.0
        eq = np.asarray(ds)[:, :, None] == np.asarray(ds)[:, None, :]
        cnt = eq.sum(-1)
        valid = ((cnt > 1) & (cnt < L)).astype(np.float32)
        fp = np.argmax(eq, axis=-1)
        pm = np.zeros((BC, L, L), np.float32)
        bi, li = np.meshgrid(np.arange(BC), np.arange(L), indexing="ij")
        pm[bi, li, fp] = 1.0
        m = dict(shared)
        imr = (images[c * BC:(c + 1) * BC]
               .reshape(BC, C, 10, PS, 10, PS)
               .transpose(1, 3, 5, 0, 2, 4).reshape(D, T))
        m.update({
            "imagesr": np.ascontiguousarray(imr),
            "ohds": ohds_a, "ohim": ohim_a, "ohpa": ohpa_a,
            "posmask": pm, "validc": np.ascontiguousarray(valid.T)})
        in_maps.append(m)

    res = run_bass_kernel_spmd(nc, in_maps, core_ids=list(range(NCORE)))
    _CACHE["last_result"] = res

    seg = np.concatenate([res.results[c]["seg_out"] for c in range(NCORE)], 0)
    closs_sum = sum(float(res.results[c]["closs_out"].sum())
                    for c in range(NCORE))
    eq = np.asarray(dataset_ids)[:, :, None] == np.asarray(dataset_ids)[:, None, :]
    cnt = eq.sum(-1)
    n_valid = int(((cnt > 1) & (cnt < L)).sum())
    c_loss = np.float32(closs_sum / max(n_valid, 1))
    topw = np.concatenate([res.results[c]["lb_out"] for c in range(NCORE)], 0)
    usage = topw.mean(axis=1)
    lb_loss = np.float32(usage.var(axis=-1, ddof=1).mean())
    return seg.astype(np.float32), c_loss, lb_loss
